# revision 1
# baseline (speedup 1.0000x reference)
"""Trainium2 Bass kernel for nn_MultiHeadAttention_54614804136658.

Forward pass of the reference collapses to: out = v + sum_h P_h[argmax_j(qh_h @ kh_h^T)]
where P_h = v @ (w_vs_h @ w_fc_h), because the straight-through estimator
(hard - stop_grad(attn) + attn) makes the forward attention an exact one-hot of
the score argmax (softmax/topk/scale are monotonic and keep the max).

Sharding: 8 cores = 2 batches x 4 head-groups (2 heads each). Each core:
  - projects qhT/khT for its 2 heads (fp32 matmuls, transposed layout from host)
  - computes P_h = v @ W_h (W fused on host), writes to DRAM scratch
  - computes scores per 128-query tile, argmax via DVE max/max_index
  - dma_gathers the argmax rows of P_h
Host: slices/transposes inputs per core, sums partial outputs + residual v.
"""
import numpy as np
from contextlib import ExitStack

B, L, E = 2, 2048, 512
H, DQK, DV = 8, 64, 256
QT = L // 128          # 16 query tiles
KBLK = 4               # key blocks of 512
ETIL = E // 128        # 4 embed tiles

_CACHE = {}


def _build(phases="ABCD", num_devices=8):
    import concourse.bass as bass
    import concourse.tile as tile
    from concourse import bacc, mybir

    F32 = mybir.dt.float32
    I16 = mybir.dt.int16
    U32 = mybir.dt.uint32

    nc = bacc.Bacc("TRN2", target_bir_lowering=False, debug=False, num_devices=num_devices)
    dbg = num_devices == 1

    qt_d = nc.dram_tensor("qt", [E, L], F32, kind="ExternalInput").ap()
    kt_d = nc.dram_tensor("kt", [E, L], F32, kind="ExternalInput").ap()
    vt_d = nc.dram_tensor("vt", [DV, L], F32, kind="ExternalInput").ap()
    wq_d = nc.dram_tensor("wq", [E, 128], F32, kind="ExternalInput").ap()
    wk_d = nc.dram_tensor("wk", [E, 128], F32, kind="ExternalInput").ap()
    W_d = nc.dram_tensor("W", [2, DV, DV], F32, kind="ExternalInput").ap()
    out_d = nc.dram_tensor("out", [2, L, DV], F32, kind="ExternalOutput").ap()
    pscr = nc.dram_tensor("pscr", [2, L, DV], F32,
                          kind="ExternalOutput" if dbg else "Internal").ap()
    iscr = nc.dram_tensor("iscr", [2, L], I16,
                          kind="ExternalOutput" if dbg else "Internal").ap()
    if dbg:
        qhT_d = nc.dram_tensor("qhT_dbg", [128, L], F32, kind="ExternalOutput").ap()
        khT_d = nc.dram_tensor("khT_dbg", [128, L], F32, kind="ExternalOutput").ap()
        idxw_d = nc.dram_tensor("idxw_dbg", [128, 128], I16, kind="ExternalOutput").ap()

    with tile.TileContext(nc) as tc, ExitStack() as ctx:
        keep = ctx.enter_context(tc.tile_pool(name="keep", bufs=1))
        qhT = keep.tile([128, L], F32, tag="qhT")   # heads stacked 64+64
        khT = keep.tile([128, L], F32, tag="khT")
        P_s = keep.tile([128, 2, QT, DV], F32, tag="P")
        idx16 = keep.tile([128, 2, QT], I16, tag="idx16")

        # ---------- phase A: q/k projections (transposed) ----------
        with tc.tile_pool(name="ldA", bufs=1) as ldA, \
             tc.tile_pool(name="psA", bufs=1, space="PSUM") as psA:
            wq_s = ldA.tile([128, ETIL, 128], F32, tag="wq")
            nc.sync.dma_start(wq_s[:], wq_d.rearrange("(t p) m -> p t m", p=128))
            wk_s = ldA.tile([128, ETIL, 128], F32, tag="wk")
            nc.sync.dma_start(wk_s[:], wk_d.rearrange("(t p) m -> p t m", p=128))
            qt_s = ldA.tile([128, ETIL, L], F32, tag="qt")
            kt_s = ldA.tile([128, ETIL, L], F32, tag="kt")
            for et in range(ETIL):
                nc.sync.dma_start(
                    qt_s[:, et, :], qt_d[et * 128:(et + 1) * 128, :])
                nc.sync.dma_start(
                    kt_s[:, et, :], kt_d[et * 128:(et + 1) * 128, :])

            for dst, w_s, x_s in ((qhT, wq_s, qt_s), (khT, wk_s, kt_s)):
                pss = []
                for nb in range(4):
                    ps_nb = psA.tile([128, 512], F32, tag=f"psA{nb}", name=f"psA{nb}")
                    pss.append(ps_nb)
                for et in range(ETIL):
                    for nb in range(4):
                        nc.tensor.matmul(
                            pss[nb][:], w_s[:, et, :],
                            x_s[:, et, nb * 512:(nb + 1) * 512],
                            start=(et == 0), stop=(et == ETIL - 1))
                for nb in range(4):
                    nc.scalar.copy(dst[:, nb * 512:(nb + 1) * 512], pss[nb][:])

        if dbg:
            nc.sync.dma_start(qhT_d, qhT[:])
            nc.sync.dma_start(khT_d, khT[:])

        # ---------- phase B+C interleaved: scores+argmax with P matmuls ----------
        if "C" in phases:
          with tc.tile_pool(name="ldB", bufs=1) as ldB, \
               tc.tile_pool(name="scps", bufs=2, space="PSUM") as scps, \
               tc.tile_pool(name="scsb", bufs=4) as scsb:
            vt_s = ldB.tile([128, 2, L], F32, tag="vt")
            nc.sync.dma_start(vt_s[:], vt_d.rearrange("(t p) n -> p t n", p=128))
            W_s = ldB.tile([128, 2, 2, DV], F32, tag="W")
            nc.sync.dma_start(W_s[:], W_d.rearrange("h (t p) m -> p h t m", p=128))
            for t in range(QT):
                for h in range(2):
                    ps = scps.tile([128, L], F32, tag="sc", name="ps_sc")
                    for kb in range(KBLK):
                        nc.tensor.matmul(
                            ps[:, kb * 512:(kb + 1) * 512],
                            qhT[h * 64:(h + 1) * 64, t * 128:(t + 1) * 128],
                            khT[h * 64:(h + 1) * 64, kb * 512:(kb + 1) * 512],
                            start=True, stop=True)
                    m8 = scsb.tile([128, 8], F32, tag="m8")
                    nc.vector.max(m8[:], ps[:])
                    i8 = scsb.tile([128, 8], U32, tag="i8")
                    nc.vector.max_index(i8[:], m8[:], ps[:])
                    nc.vector.tensor_copy(idx16[:, h, t:t + 1], i8[:, 0:1])
                # interleave P-projection work (phase B): 2 key-blocks per t
                for j in range(2):
                    kb = (t * 2 + j)
                    h2, kb2 = divmod(kb, QT)
                    psb = scps.tile([128, DV], F32, tag="sc", name="ps_pb")
                    for et in range(2):
                        nc.tensor.matmul(
                            psb[:], vt_s[:, et, kb2 * 128:(kb2 + 1) * 128],
                            W_s[:, h2, et, :],
                            start=(et == 0), stop=(et == 1))
                    nc.scalar.copy(P_s[:, h2, kb2, :], psb[:])
                    if kb2 == QT - 1:
                        nc.sync.dma_start(
                            pscr[h2].rearrange("(t p) e -> p t e", p=128),
                            P_s[:, h2])
        # ---------- phase D: index roundtrip, gather, output ----------
        if "D" in phases:
          sub = phases.split("D", 1)[1] or "123"
          with tc.tile_pool(name="gth", bufs=1) as gth:
            for h in range(2):
                nc.sync.dma_start(
                    iscr[h].rearrange("(t p) -> p t", p=128), idx16[:, h])
            for h in range(2):
                if "2" not in sub:
                    break
                idxw = gth.tile([128, 128], I16, tag="idxw")
                for r in range(8):
                    nc.sync.dma_start(
                        idxw[16 * r:16 * (r + 1), :],
                        iscr[h].rearrange("(c p) -> p c", p=16))
                if dbg and h == 0:
                    nc.sync.dma_start(idxw_d, idxw[:])
                if "3" not in sub:
                    continue
                g = gth.tile([128, QT, DV], F32, tag="gout")
                nc.gpsimd.dma_gather(
                    out_ap=g[:], in_ap=pscr[h], idxs_ap=idxw[:],
                    num_idxs=L, num_idxs_reg=L, elem_size=DV,
                    single_packet=False)
                nc.sync.dma_start(
                    out_d[h].rearrange("(t p) e -> p t e", p=128), g[:])

    nc.compile()
    return nc


def kernel(**inputs):
    from concourse.bass_utils import run_bass_kernel_spmd

    q = np.asarray(inputs["q"], np.float32)
    k = np.asarray(inputs["k"], np.float32)
    v = np.asarray(inputs["v"], np.float32)
    w_qs = np.asarray(inputs["w_qs"], np.float32)
    w_ks = np.asarray(inputs["w_ks"], np.float32)
    w_vs = np.asarray(inputs["w_vs"], np.float32)
    w_fc = np.asarray(inputs["w_fc"], np.float32)

    if "nc" not in _CACHE:
        _CACHE["nc"] = _build()
    nc = _CACHE["nc"]

    # fused per-head value->output projection
    W = np.empty((H, DV, DV), np.float32)
    for h in range(H):
        W[h] = (w_vs[:, h * DV:(h + 1) * DV].astype(np.float64)
                @ w_fc[h * DV:(h + 1) * DV, :].astype(np.float64)).astype(np.float32)

    in_maps = []
    for c in range(8):
        b, g = divmod(c, 4)
        in_maps.append({
            "qt": np.ascontiguousarray(q[b].T),
            "kt": np.ascontiguousarray(k[b].T),
            "vt": np.ascontiguousarray(v[b].T),
            "wq": np.ascontiguousarray(w_qs[:, g * 128:(g + 1) * 128]),
            "wk": np.ascontiguousarray(w_ks[:, g * 128:(g + 1) * 128]),
            "W": np.ascontiguousarray(W[2 * g:2 * g + 2]),
        })

    res = run_bass_kernel_spmd(nc, in_maps, core_ids=list(range(8)))
    _CACHE["last_result"] = res

    out = np.array(v)  # residual
    for c in range(8):
        b = c // 4
        co = res.results[c]["out"]
        out[b] += co[0]
        out[b] += co[1]
    return out



# revision 2
# speedup vs baseline: 1.4458x; 1.4458x over previous
"""Trainium2 Bass kernel for nn_MultiHeadAttention_54614804136658.

Forward pass of the reference collapses to: out = v + sum_h P_h[argmax_j(qh_h @ kh_h^T)]
where P_h = v @ (w_vs_h @ w_fc_h), because the straight-through estimator
(hard - stop_grad(attn) + attn) makes the forward attention an exact one-hot of
the score argmax (softmax/topk/scale are monotonic and keep the max).

Sharding: 8 cores = 2 batches x 4 head-groups (2 heads each). Per core:
  phase B (first, overlaps input DMA): P_h = v @ W_h in bf16 -> pscr (DRAM, bf16)
  phase A: khT fp32 projection; qhT projected per-tile inside the main loop
  steady loop over 32 (h,t) score tiles:
    PE: 2 half-tile fp32 matmuls [128,1024] into PSUM (+ next qhT projection)
    ACT: PSUM -> SBUF copy (frees PSUM quickly)
    DVE: max8 + max_index over [128,2048] SBUF + idx16 write
  phase D: per head, index roundtrip via DRAM, dma_gather of P rows (bf16),
    h0 gather overlaps h1 compute; h1 gathered in 2 halves to cut the tail.
Host: fuses W = w_vs_h @ w_fc_h, transposes/slices inputs, sums partials + v.
"""
import numpy as np
from contextlib import ExitStack

B, L, E = 2, 2048, 512
H, DQK, DV = 8, 64, 256
QT = L // 128           # 16 query tiles
ETIL = E // 128         # 4 embed tiles

_CACHE = {}


def _build(phases="ABCD", num_devices=8):
    import concourse.bass as bass
    import concourse.tile as tile
    from concourse import bacc, mybir

    F32 = mybir.dt.float32
    BF16 = mybir.dt.bfloat16
    I16 = mybir.dt.int16
    U32 = mybir.dt.uint32

    nc = bacc.Bacc("TRN2", target_bir_lowering=False, debug=False,
                   num_devices=num_devices)
    dbg = num_devices == 1

    qt_d = nc.dram_tensor("qt", [E, L], F32, kind="ExternalInput").ap()
    kt_d = nc.dram_tensor("kt", [E, L], F32, kind="ExternalInput").ap()
    vt_d = nc.dram_tensor("vt", [DV, L], BF16, kind="ExternalInput").ap()
    wq_d = nc.dram_tensor("wq", [E, 128], F32, kind="ExternalInput").ap()
    wk_d = nc.dram_tensor("wk", [E, 128], F32, kind="ExternalInput").ap()
    W_d = nc.dram_tensor("W", [2, DV, DV], BF16, kind="ExternalInput").ap()
    out_d = nc.dram_tensor("out", [2, L, DV], BF16, kind="ExternalOutput").ap()
    pscr = nc.dram_tensor("pscr", [2, L, DV], BF16,
                          kind="ExternalOutput" if dbg else "Internal").ap()
    iscr = nc.dram_tensor("iscr", [2, L], I16,
                          kind="ExternalOutput" if dbg else "Internal").ap()

    with tile.TileContext(nc) as tc, ExitStack() as ctx:
        keep = ctx.enter_context(tc.tile_pool(name="keep", bufs=1))
        qhT = keep.tile([128, L], F32, tag="qhT")   # 2 heads stacked 64+64
        khT = keep.tile([128, L], F32, tag="khT")
        idx16 = keep.tile([128, 2, QT], I16, tag="idx16")

        # ---------- input DMAs (vt first: B phase starts earliest) ----------
        ldB = ctx.enter_context(tc.tile_pool(name="ldB", bufs=1))
        vt_s = ldB.tile([128, 2, L], BF16, tag="vt")
        nc.sync.dma_start(vt_s[:], vt_d.rearrange("(t p) n -> p t n", p=128))
        W_s = ldB.tile([128, 2, 2, DV], BF16, tag="W")
        nc.sync.dma_start(W_s[:], W_d.rearrange("h (t p) m -> p h t m", p=128))

        ldK = ctx.enter_context(tc.tile_pool(name="ldK", bufs=1))
        wk_s = ldK.tile([128, ETIL, 128], F32, tag="wk")
        nc.sync.dma_start(wk_s[:], wk_d.rearrange("(t p) m -> p t m", p=128))
        kt_s = ldK.tile([128, ETIL, L], F32, tag="kt")
        for et in range(ETIL):
            nc.sync.dma_start(kt_s[:, et, :], kt_d[et * 128:(et + 1) * 128, :])

        ldQ = ctx.enter_context(tc.tile_pool(name="ldQ", bufs=1))
        wq_s = ldQ.tile([128, ETIL, 128], F32, tag="wq")
        nc.sync.dma_start(wq_s[:], wq_d.rearrange("(t p) m -> p t m", p=128))
        qt_s = ldQ.tile([128, ETIL, L], F32, tag="qt")
        for et in range(ETIL):
            nc.sync.dma_start(qt_s[:, et, :], qt_d[et * 128:(et + 1) * 128, :])

        # ---------- phase B: P_h = v @ W_h (bf16) ----------
        P_s = keep.tile([128, 2, QT, DV], BF16, tag="P")
        with tc.tile_pool(name="psB", bufs=2, space="PSUM") as psB:
            for h in range(2):
                for rt in range(QT):
                    psb = psB.tile([128, DV], F32, tag="pb", name="ps_pb")
                    for et in range(2):
                        nc.tensor.matmul(
                            psb[:], vt_s[:, et, rt * 128:(rt + 1) * 128],
                            W_s[:, h, et, :], start=(et == 0), stop=(et == 1))
                    nc.scalar.copy(P_s[:, h, rt, :], psb[:])
                nc.sync.dma_start(
                    pscr[h].rearrange("(t p) e -> p t e", p=128), P_s[:, h])

        # ---------- phase A: khT projection (fp32) ----------
        with tc.tile_pool(name="psA", bufs=1, space="PSUM") as psA:
            for nb in range(4):
                ps_nb = psA.tile([128, 512], F32, tag=f"psA{nb}", name=f"psA{nb}")
                for et in range(ETIL):
                    nc.tensor.matmul(
                        ps_nb[:], wk_s[:, et, :],
                        kt_s[:, et, nb * 512:(nb + 1) * 512],
                        start=(et == 0), stop=(et == ETIL - 1))
                nc.scalar.copy(khT[:, nb * 512:(nb + 1) * 512], ps_nb[:])

        # ---------- steady loop: scores + argmax; qhT projected JIT ----------
        if "C" in phases:
          with tc.tile_pool(name="scps", bufs=3, space="PSUM") as scps, \
               tc.tile_pool(name="qps", bufs=2, space="PSUM") as qps, \
               tc.tile_pool(name="ysb", bufs=2) as ysb, \
               tc.tile_pool(name="scsb", bufs=4) as scsb, \
               tc.tile_pool(name="gth", bufs=1) as gth:

            def project_q(t):
                psq = qps.tile([128, 128], F32, tag="q", name="ps_q")
                for et in range(ETIL):
                    nc.tensor.matmul(
                        psq[:], wq_s[:, et, :],
                        qt_s[:, et, t * 128:(t + 1) * 128],
                        start=(et == 0), stop=(et == ETIL - 1))
                nc.scalar.copy(qhT[:, t * 128:(t + 1) * 128], psq[:])

            def gather(h, part, nparts):
                # part covers QT//nparts query tiles of head h
                tpp = QT // nparts
                n_idx = tpp * 128
                nc.sync.dma_start(
                    iscr[h].rearrange("(t p) -> p t", p=128)[:, part * tpp:(part + 1) * tpp],
                    idx16[:, h, part * tpp:(part + 1) * tpp])
                idxw = gth.tile([128, n_idx // 16], I16, tag=f"idxw{h}_{part}")
                for r in range(8):
                    nc.sync.dma_start(
                        idxw[16 * r:16 * (r + 1), :],
                        iscr[h][part * n_idx:(part + 1) * n_idx]
                        .rearrange("(c p) -> p c", p=16))
                g = gth.tile([128, tpp, DV], BF16, tag=f"g{h}_{part}")
                nc.gpsimd.dma_gather(
                    out_ap=g[:], in_ap=pscr[h], idxs_ap=idxw[:],
                    num_idxs=n_idx, num_idxs_reg=n_idx, elem_size=DV,
                    single_packet=False)
                nc.sync.dma_start(
                    out_d[h].rearrange("(t p) e -> p t e", p=128)
                    [:, part * tpp:(part + 1) * tpp, :], g[:])

            project_q(0)
            for h in range(2):
                for t in range(QT):
                    # next qhT projection rides along with h0's tiles
                    if h == 0 and t < QT - 1:
                        project_q(t + 1)
                    y = ysb.tile([128, L], F32, tag="y", name="y")
                    for half in range(2):
                        ps = scps.tile([128, 1024], F32, tag="sc", name="ps_sc")
                        for kb in range(2):
                            col = half * 1024 + kb * 512
                            nc.tensor.matmul(
                                ps[:, kb * 512:(kb + 1) * 512],
                                qhT[h * 64:(h + 1) * 64, t * 128:(t + 1) * 128],
                                khT[h * 64:(h + 1) * 64, col:col + 512],
                                start=True, stop=True)
                        nc.scalar.copy(y[:, half * 1024:(half + 1) * 1024], ps[:])
                    m8 = scsb.tile([128, 8], F32, tag="m8")
                    nc.vector.max(m8[:], y[:])
                    i8 = scsb.tile([128, 8], U32, tag="i8")
                    nc.vector.max_index(i8[:], m8[:], y[:])
                    nc.vector.tensor_copy(idx16[:, h, t:t + 1], i8[:, 0:1])
                if "D" in phases:
                    if h == 0:
                        gather(0, 0, 1)     # whole head while h1 computes
            if "D" in phases:
                gather(1, 0, 2)
                gather(1, 1, 2)

    nc.compile()
    return nc


def kernel(**inputs):
    from concourse.bass_utils import run_bass_kernel_spmd

    q = np.asarray(inputs["q"], np.float32)
    k = np.asarray(inputs["k"], np.float32)
    v = np.asarray(inputs["v"], np.float32)
    w_qs = np.asarray(inputs["w_qs"], np.float32)
    w_ks = np.asarray(inputs["w_ks"], np.float32)
    w_vs = np.asarray(inputs["w_vs"], np.float32)
    w_fc = np.asarray(inputs["w_fc"], np.float32)

    if "nc" not in _CACHE:
        _CACHE["nc"] = _build()
    nc = _CACHE["nc"]

    import ml_dtypes
    bf16 = ml_dtypes.bfloat16

    # fused per-head value->output projection
    W = np.empty((H, DV, DV), np.float32)
    for h in range(H):
        W[h] = (w_vs[:, h * DV:(h + 1) * DV].astype(np.float64)
                @ w_fc[h * DV:(h + 1) * DV, :].astype(np.float64)).astype(np.float32)

    in_maps = []
    for c in range(8):
        b, g = divmod(c, 4)
        in_maps.append({
            "qt": np.ascontiguousarray(q[b].T),
            "kt": np.ascontiguousarray(k[b].T),
            "vt": np.ascontiguousarray(v[b].T).astype(bf16),
            "wq": np.ascontiguousarray(w_qs[:, g * 128:(g + 1) * 128]),
            "wk": np.ascontiguousarray(w_ks[:, g * 128:(g + 1) * 128]),
            "W": np.ascontiguousarray(W[2 * g:2 * g + 2]).astype(bf16),
        })

    res = run_bass_kernel_spmd(nc, in_maps, core_ids=list(range(8)))
    _CACHE["last_result"] = res

    out = np.array(v)  # residual
    for c in range(8):
        b = c // 4
        co = res.results[c]["out"]
        out[b] += np.asarray(co[0], np.float32)
        out[b] += np.asarray(co[1], np.float32)
    return out
